# revision 6
# baseline (speedup 1.0000x reference)
"""Trainium2 Bass kernel for nn_BinarizeLayer (chain Viterbi binarization).

Algorithm
---------
The reference is a 2-state Viterbi DP over an 8.4M-node chain.  Writing
d_i = a0_i - a1_i (score difference of the two states), the forward pass
collapses to the scalar recurrence

    d_i = e_i + clamp(d_{i-1}, -lam, lam),        e_i = 2*p_i - 1,

and the backtracking pass to

    label_{i-1} = label_i ? (d_{i-1} >= -lam) : (d_{i-1} > lam).

Conjugating the clamp recurrence by prefix sums (s = running sum of -e,
w = d - (-s) + lam) turns it into

    w_k = min(max(w_{k-1}, sbar_{k-1}), sbar_{k-1} + 2*lam),

which is exactly the hardware `tensor_tensor_scan` (op0=max, op1=min).
The backtracking pass is a reversed scan with op0=logical_and,
op1=logical_or over precomputed threshold bits
    gt_k = (d_k > lam)  <=>  w_k > sbar_k + 2*lam
    ge_k = (d_k >= -lam) <=> w_k >= sbar_k.

Engine split (per core): the passes are spread across three engines so
the DVE (the only engine that can run scans in this build) does little
besides the three serial recurrences:
  * ACT : ebar = 1-2p, SP = SB + 2*lam            (affine, 1-input)
  * DVE : SB scan, W clamp-walk scan, bwd scan    (serial recurrences)
          GT = DP > 2*lam      (tensor_scalar, 2 elem/cycle fp32)
  * Pool: DP = w - SB_incl     (tensor_tensor subtract)
          GE = DP >= 0         (tensor_scalar)

Sharding: the chain is split into 8 core slices x 128 partition rows of
8192 payload elements, each row padded with a 64-element halo on both
sides (clamp recurrences forget their initial state as soon as the walk
saturates a clamp bound, so a 64-step warm-up reproduces the exact
sequential fp32 state; validated against the sequential reference).
The global chain ends are padded with p=0.5 (e=0 exactly), which makes
the boundary conditions exact; the final-label boundary condition is
injected by overwriting gt/ge at the last halo column with (d > 0).
"""

import numpy as np

import concourse.bass as bass
import concourse.mybir as mybir
from concourse import tile
from concourse import bass_utils

LAM = 0.75
N = 8388608
NCORES = 8
P = 128          # partitions
W = 64           # halo / warm-up width
D = 8192         # payload elements per partition row
R = D + 2 * W    # row length incl. halos
FWD_BLOCKS = [1024, 1024, 2048, 2048, 2048]   # payload col blocks (sum = D)
assert sum(FWD_BLOCKS) == D
BWD_ELL = 2048   # backward block width
NBWD = D // BWD_ELL


def _build():
    f32 = mybir.dt.float32
    i8 = mybir.dt.int8
    Alu = mybir.AluOpType
    Copy = mybir.ActivationFunctionType.Copy

    nc = bass.Bass()
    x = nc.dram_tensor("x", [P, R], f32, kind="ExternalInput")
    y = nc.dram_tensor("y", [P, D], i8, kind="ExternalOutput")

    with tile.TileContext(nc) as tc:
        with tc.tile_pool(name="big", bufs=1) as big, \
             tc.tile_pool(name="dp", bufs=2) as dpp:
            XT = big.tile([P, R], f32)        # input p, then ebar = 1-2p in place
            SB = big.tile([P, R + 1], f32)    # running sum of ebar; col0 = 0
            SP = big.tile([P, R + 1], f32)    # SB + 2*lam
            WT = XT                           # clamp walk overwrites consumed ebar
            GT = big.tile([P, R], f32)        # d > lam
            GE = big.tile([P, R], f32)        # d >= -lam
            LB = big.tile([P, R], i8)         # labels
            TMP = big.tile([P, 1], f32)

            nc.vector.memset(SB[:, 0:1], 0.0)
            nc.vector.memset(SP[:, 0:1], 2 * LAM)

            blocks = []
            c = 0
            for bw in FWD_BLOCKS:
                blocks.append((c, bw))
                c += bw
            blocks.append((D, 2 * W))
            for (c0, bw) in blocks:
                nc.sync.dma_start(XT[:, c0:c0 + bw], x[:, c0:c0 + bw])
                # ebar = 1 - 2p (in place)
                nc.scalar.activation(XT[:, c0:c0 + bw], XT[:, c0:c0 + bw],
                                     Copy, bias=1.0, scale=-2.0)
                # chained running sum: SB[c+1] = SB[c] + ebar[c]
                init = 0.0 if c0 == 0 else SB[:, c0:c0 + 1]
                nc.vector.tensor_tensor_scan(
                    SB[:, c0 + 1:c0 + 1 + bw], XT[:, c0:c0 + bw],
                    XT[:, c0:c0 + bw], init, Alu.add, Alu.bypass)
                nc.scalar.activation(SP[:, c0 + 1:c0 + 1 + bw],
                                     SB[:, c0 + 1:c0 + 1 + bw],
                                     Copy, bias=2 * LAM)
                # chained clamp walk: w = min(max(w, SB_excl), SP_excl)
                winit = LAM if c0 == 0 else WT[:, c0 - 1:c0]
                nc.vector.tensor_tensor_scan(
                    WT[:, c0:c0 + bw], SB[:, c0:c0 + bw],
                    SP[:, c0:c0 + bw], winit, Alu.max, Alu.min)
                # DP = w - SB_incl = d + lam (Pool), then threshold bits:
                # GE = DP >= 0 (Pool), GT = DP > 2*lam (DVE, 2x tensor_scalar)
                DP = dpp.tile([P, bw], f32)
                nc.gpsimd.tensor_tensor(DP[:], WT[:, c0:c0 + bw],
                                        SB[:, c0 + 1:c0 + 1 + bw],
                                        Alu.subtract)
                nc.gpsimd.tensor_scalar(GE[:, c0:c0 + bw], DP[:], 0.0, None,
                                        Alu.is_ge)
                nc.vector.tensor_scalar(GT[:, c0:c0 + bw], DP[:], 2 * LAM,
                                        None, Alu.is_gt)

            # boundary sentinel at the last halo column: gt = ge = (d > 0)
            nc.scalar.activation(TMP[:], SB[:, R:R + 1], Copy, bias=LAM)
            nc.vector.tensor_tensor(GT[:, R - 1:R], WT[:, R - 1:R], TMP[:],
                                    Alu.is_gt)
            nc.vector.tensor_tensor(GE[:, R - 1:R], WT[:, R - 1:R], TMP[:],
                                    Alu.is_gt)

            # backtracking: reversed logical scan per block with W warm-up
            for s in range(NBWD):
                c0 = W + s * BWD_ELL
                wd = BWD_ELL + W
                nc.vector.tensor_tensor_scan(
                    LB[:, c0:c0 + wd][:, ::-1],
                    GE[:, c0:c0 + wd][:, ::-1],
                    GT[:, c0:c0 + wd][:, ::-1],
                    0.0, Alu.logical_and, Alu.logical_or)
                nc.sync.dma_start(y[:, c0 - W:c0 - W + BWD_ELL],
                                  LB[:, c0:c0 + BWD_ELL])
    return nc


def _legalize_waits(nc, limit=1):
    """Split instructions carrying more than `limit` sem-waits.

    This walrus build rejects instructions whose sync_info has more wait
    commands than the ISA encoding allows (Tile can accumulate several).
    Excess waits move onto NoOps prepended on the same engine, which
    preserves per-engine ordering semantics.
    """
    import concourse.mybir as mybir
    for fn in nc.m.functions:
        for blk in fn.blocks:
            insts = blk.instructions
            i = 0
            while i < len(insts):
                inst = insts[i]
                si = getattr(inst, "sync_info", None)
                if si is not None and si.on_wait and len(si.on_wait) > limit:
                    waits = list(si.on_wait)
                    inst.sync_info = mybir.SyncInfo(
                        on_wait=waits[-limit:], on_update=list(si.on_update))
                    pending = waits[:-limit]
                    for j in range(0, len(pending), limit):
                        nop = mybir.InstNoOp(
                            name=nc.get_next_instruction_name(),
                            sync_info=mybir.SyncInfo(
                                on_wait=pending[j:j + limit], on_update=[]),
                            bass_nofuse=True,
                            engine=inst.engine,
                        )
                        insts.insert(i, nop)
                        i += 1
                i += 1
    return nc


_nc_cache = None


def _get_nc():
    global _nc_cache
    if _nc_cache is None:
        _nc_cache = _legalize_waits(_build())
    return _nc_cache


def _shard(inputs: np.ndarray):
    p = np.ascontiguousarray(inputs, dtype=np.float32)
    assert p.shape == (N,)
    pad = np.full(W, 0.5, np.float32)
    pp = np.concatenate([pad, p, pad])
    nrows = N // D
    X = np.lib.stride_tricks.as_strided(pp, (nrows, R), (D * 4, 4))
    return [{"x": np.ascontiguousarray(X[k * P:(k + 1) * P])}
            for k in range(NCORES)]


def _run(inputs: np.ndarray, trace: bool = False):
    in_maps = _shard(inputs)
    res = bass_utils.run_bass_kernel_spmd(_get_nc(), in_maps,
                                          core_ids=list(range(NCORES)),
                                          trace=trace)
    lab = np.concatenate([np.asarray(res.results[k]["y"]).reshape(-1)
                          for k in range(NCORES)])
    return lab.astype(np.int32), res


def kernel(inputs: np.ndarray) -> np.ndarray:
    lab, _ = _run(inputs, trace=False)
    return lab


# revision 7
# speedup vs baseline: 2.4475x; 2.4475x over previous
"""Trainium2 Bass kernel for nn_BinarizeLayer (chain Viterbi binarization).

Algorithm
---------
The reference is a 2-state Viterbi DP over an 8.4M-node chain.  Writing
d_i = a0_i - a1_i (score difference of the two states), the forward pass
collapses to the scalar recurrence

    d_i = e_i + clamp(d_{i-1}, -lam, lam),        e_i = 2*p_i - 1,

and the backtracking pass to

    label_{i-1} = label_i ? (d_{i-1} >= -lam) : (d_{i-1} > lam).

The whole problem is rescaled by 1/(2*lam) (argmins are scale
invariant), so lam~ = 0.5 and the transition cost 2*lam~ = 1 exactly.
Conjugating the clamp recurrence by prefix sums (SB = running sum of
(1-2p)/(2 lam), w = d~ + SB_incl + lam~) turns the forward pass into

    w_k = min(max(w_{k-1}, SB_k), SB_k + 1),

which is exactly the hardware `tensor_tensor_scan` (op0=max, op1=min)
over (SB, SP = SB + 1).  The backtracking pass becomes a single scan
too: label_{k-1} = [d~ > lam~ - label_k] = [w + label_k > SP_incl]
(using that the state shift is exactly 1), i.e. a reversed
`tensor_tensor_scan` with op0=add, op1=is_gt over (w, SP_incl) — no
threshold bitmaps needed at all.  (The state=1 threshold becomes
strictly-greater instead of >=, which differs only on exact fp ties.)

Engine split (per core):
  * ACT : ebar = (1-2p)/(2 lam), SP = SB + 1        (affine, 1-input)
  * DVE : SB scan, W clamp-walk scan, bwd label scan
  * Pool: unused (GpSimd streaming ops measure ~12x slower than DVE on
    this hardware and contend with DVE for SBUF ports)

Sharding: the chain is split into 8 core slices x 128 partition rows of
8192 payload elements, each row padded with a 64-element halo on both
sides (clamp recurrences forget their initial state as soon as the walk
saturates a clamp bound, so a 64-step warm-up reproduces the exact
sequential fp32 state).  The global chain ends are padded with p=0.5
(ebar = 0 exactly, incl. the rescaling, since fl(4/3)/2 == fl(2/3)).
The final-label boundary condition is injected by overwriting w at the
last halo column with +-1e38 by the sign of d~ there; backward blocks
chain exactly (descending emission, init = label just computed by the
block to the right), so backtracking is exact.
"""

import numpy as np

import concourse.bass as bass
import concourse.mybir as mybir
from concourse import tile
from concourse import bass_utils

LAM = 0.75
N = 8388608
NCORES = 8
P = 128          # partitions
W = 64           # halo / warm-up width
D = 8192         # payload elements per partition row
R = D + 2 * W    # row length incl. halos
BWD_ELL = 2048   # backward block width
NBWD = D // BWD_ELL
FWD_BLOCKS = [(0, 2048), (2048, 2048), (4096, 2048), (6144, 2176)]


def _build():
    f32 = mybir.dt.float32
    i8 = mybir.dt.int8
    Alu = mybir.AluOpType
    Copy = mybir.ActivationFunctionType.Copy

    nc = bass.Bass()
    x = nc.dram_tensor("x", [P, R], f32, kind="ExternalInput")
    y = nc.dram_tensor("y", [P, D], i8, kind="ExternalOutput")

    with tile.TileContext(nc) as tc:
        with tc.tile_pool(name="big", bufs=1) as big:
            XT = big.tile([P, R], f32)        # input p, then ebar in place
            SB = big.tile([P, R + 1], f32)    # running sum of ebar; col0 = 0
            SP = big.tile([P, R + 1], f32)    # SB + 1
            WT = XT                           # clamp walk overwrites consumed ebar
            LB = big.tile([P, R], i8)         # labels
            TMP = big.tile([P, 1], f32)
            CB = big.tile([P, 1], f32)

            nc.vector.memset(SB[:, 0:1], 0.0)
            nc.vector.memset(SP[:, 0:1], 1.0)

            for (c0, bw) in FWD_BLOCKS:
                nc.sync.dma_start(XT[:, c0:c0 + bw], x[:, c0:c0 + bw])
                # ebar = (1 - 2p) / (2 lam)  (in place)
                nc.scalar.activation(XT[:, c0:c0 + bw], XT[:, c0:c0 + bw],
                                     Copy, bias=1.0 / (2 * LAM),
                                     scale=-2.0 / (2 * LAM))
                # chained running sum: SB[c+1] = SB[c] + ebar[c]
                init = 0.0 if c0 == 0 else SB[:, c0:c0 + 1]
                nc.vector.tensor_tensor_scan(
                    SB[:, c0 + 1:c0 + 1 + bw], XT[:, c0:c0 + bw],
                    XT[:, c0:c0 + bw], init, Alu.add, Alu.bypass)
                nc.scalar.activation(SP[:, c0 + 1:c0 + 1 + bw],
                                     SB[:, c0 + 1:c0 + 1 + bw],
                                     Copy, bias=1.0)
                # chained clamp walk: w = min(max(w, SB_excl), SP_excl)
                winit = 0.5 if c0 == 0 else WT[:, c0 - 1:c0]
                nc.vector.tensor_tensor_scan(
                    WT[:, c0:c0 + bw], SB[:, c0:c0 + bw],
                    SP[:, c0:c0 + bw], winit, Alu.max, Alu.min)

            # boundary sentinel at the last halo column:
            # w[R-1] := +-1e38 by the sign of d~ = w - SB_incl - lam~ there
            nc.scalar.activation(TMP[:], SB[:, R:R + 1], Copy, bias=0.5)
            nc.vector.tensor_tensor(CB[:], WT[:, R - 1:R], TMP[:], Alu.is_gt)
            nc.scalar.activation(WT[:, R - 1:R], CB[:], Copy,
                                 scale=2e38, bias=-1e38)

            # backtracking: label' = [w + label > SP_incl], reversed scan.
            # Emitted right-to-left; each block seeds from the label the
            # previous (righter) block produced, so chaining is exact.
            for s in range(NBWD - 1, -1, -1):
                c0 = W + s * BWD_ELL
                wd = BWD_ELL + (W if s == NBWD - 1 else 0)
                init = 0.0 if s == NBWD - 1 else LB[:, c0 + wd:c0 + wd + 1]
                nc.vector.tensor_tensor_scan(
                    LB[:, c0:c0 + wd][:, ::-1],
                    WT[:, c0:c0 + wd][:, ::-1],
                    SP[:, c0 + 1:c0 + 1 + wd][:, ::-1],
                    init, Alu.add, Alu.is_gt)
                nc.sync.dma_start(y[:, c0 - W:c0 - W + BWD_ELL],
                                  LB[:, c0:c0 + BWD_ELL])
    return nc


def _legalize_waits(nc, limit=1):
    """Split instructions carrying more than `limit` sem-waits.

    This walrus build rejects instructions whose sync_info has more wait
    commands than the ISA encoding allows (Tile can accumulate several).
    Excess waits move onto NoOps prepended on the same engine, which
    preserves per-engine ordering semantics.
    """
    import concourse.mybir as mybir
    for fn in nc.m.functions:
        for blk in fn.blocks:
            insts = blk.instructions
            i = 0
            while i < len(insts):
                inst = insts[i]
                si = getattr(inst, "sync_info", None)
                if si is not None and si.on_wait and len(si.on_wait) > limit:
                    waits = list(si.on_wait)
                    inst.sync_info = mybir.SyncInfo(
                        on_wait=waits[-limit:], on_update=list(si.on_update))
                    pending = waits[:-limit]
                    for j in range(0, len(pending), limit):
                        nop = mybir.InstNoOp(
                            name=nc.get_next_instruction_name(),
                            sync_info=mybir.SyncInfo(
                                on_wait=pending[j:j + limit], on_update=[]),
                            bass_nofuse=True,
                            engine=inst.engine,
                        )
                        insts.insert(i, nop)
                        i += 1
                i += 1
    return nc


_nc_cache = None


def _get_nc():
    global _nc_cache
    if _nc_cache is None:
        _nc_cache = _legalize_waits(_build())
    return _nc_cache


def _shard(inputs: np.ndarray):
    p = np.ascontiguousarray(inputs, dtype=np.float32)
    assert p.shape == (N,)
    pad = np.full(W, 0.5, np.float32)
    pp = np.concatenate([pad, p, pad])
    nrows = N // D
    X = np.lib.stride_tricks.as_strided(pp, (nrows, R), (D * 4, 4))
    return [{"x": np.ascontiguousarray(X[k * P:(k + 1) * P])}
            for k in range(NCORES)]


def _run(inputs: np.ndarray, trace: bool = False):
    in_maps = _shard(inputs)
    res = bass_utils.run_bass_kernel_spmd(_get_nc(), in_maps,
                                          core_ids=list(range(NCORES)),
                                          trace=trace)
    lab = np.concatenate([np.asarray(res.results[k]["y"]).reshape(-1)
                          for k in range(NCORES)])
    return lab.astype(np.int32), res


def kernel(inputs: np.ndarray) -> np.ndarray:
    lab, _ = _run(inputs, trace=False)
    return lab


# revision 12
# speedup vs baseline: 2.8721x; 1.1735x over previous
"""Trainium2 Bass kernel for nn_BinarizeLayer (chain Viterbi binarization).

Algorithm
---------
The reference is a 2-state Viterbi DP over an 8.4M-node chain.  Writing
d_i = a0_i - a1_i (score difference of the two states), the forward pass
collapses to the scalar recurrence

    d_i = e_i + clamp(d_{i-1}, -lam, lam),        e_i = 2*p_i - 1,

and the backtracking pass to

    label_{i-1} = label_i ? (d_{i-1} >= -lam) : (d_{i-1} > lam).

The whole problem is rescaled by 1/(2*lam) (argmins are scale
invariant), so lam~ = 0.5 and the transition cost 2*lam~ = 1 exactly.
Conjugating the clamp recurrence by prefix sums (SB = running sum of
(1-2p)/(2 lam), w = d~ + SB_incl + lam~) turns the forward pass into

    w_k = min(max(w_{k-1}, SB_k), SB_k + 1),

which is exactly the hardware `tensor_tensor_scan` (op0=max, op1=min)
over (SB, SP = SB + 1).  The backtracking pass becomes a single scan
too: label_{k-1} = [d~ > lam~ - label_k] = [w + label_k > SP_incl]
(using that the state shift is exactly 1), i.e. a reversed
`tensor_tensor_scan` with op0=add, op1=is_gt over (w, SP_incl) — no
threshold bitmaps needed at all.  (The state=1 threshold becomes
strictly-greater instead of >=, which differs only on exact fp ties.)

Engine split (per core):
  * ACT : ebar = (1-2p)/(2 lam), SP = SB + 1        (affine, 1-input)
  * DVE : SB scan, W clamp-walk scan, bwd label scan
  * Pool: unused (GpSimd streaming ops measure ~12x slower than DVE on
    this hardware and contend with DVE for SBUF ports)

Sharding: the chain is split into 8 core slices x 128 partition rows of
8192 payload elements, each row padded with a 64-element halo on both
sides (clamp recurrences forget their initial state as soon as the walk
saturates a clamp bound, so a 64-step warm-up reproduces the exact
sequential fp32 state).  The global chain ends are padded with p=0.5
(ebar = 0 exactly, incl. the rescaling, since fl(4/3)/2 == fl(2/3)).
The final-label boundary condition is injected by overwriting w at the
last halo column with +-1e38 by the sign of d~ there; backward blocks
chain exactly (descending emission, init = label just computed by the
block to the right), so backtracking is exact.
"""

import numpy as np

import concourse.bass as bass
import concourse.mybir as mybir
from concourse import tile
from concourse import bass_utils

LAM = 0.75
N = 8388608
NCORES = 8
P = 128          # partitions
W = 64           # halo / warm-up width
D = 8192         # payload elements per partition row
R = D + 2 * W    # row length incl. halos
# graded forward blocks: small first so the DVE scan chain starts ASAP
_FWD_W = [256, 512, 1024, 2048, 2048, 2432]
FWD_BLOCKS = []
_c = 0
for _w in _FWD_W:
    FWD_BLOCKS.append((_c, _w))
    _c += _w
assert _c == R
# backward blocks, emitted right-to-left; last (leftmost) small to cut the tail
_BWD_W = [512, 2048, 2048, 2048, 1536]
assert sum(_BWD_W) == D
BWD_BLOCKS = []
_c = W
for _w in _BWD_W:
    BWD_BLOCKS.append((_c, _w))
    _c += _w


def _build():
    f32 = mybir.dt.float32
    i8 = mybir.dt.int8
    Alu = mybir.AluOpType
    Copy = mybir.ActivationFunctionType.Copy

    nc = bass.Bass()
    x = nc.dram_tensor("x", [P, R], f32, kind="ExternalInput")
    y = nc.dram_tensor("y", [P, D], i8, kind="ExternalOutput")

    with tile.TileContext(nc) as tc:
        with tc.tile_pool(name="big", bufs=1) as big:
            XT = big.tile([P, R], f32)        # input p, then ebar in place
            SB = big.tile([P, R + 1], f32)    # running sum of ebar; col0 = 0
            SP = big.tile([P, R + 1], f32)    # SB + 1
            WT = XT                           # clamp walk overwrites consumed ebar
            LB = big.tile([P, R], i8)         # labels
            TMP = big.tile([P, 1], f32)
            CB = big.tile([P, 1], f32)

            nc.vector.memset(SB[:, 0:1], 0.0)
            nc.vector.memset(SP[:, 0:1], 1.0)

            for bi, (c0, bw) in enumerate(FWD_BLOCKS):
                nc.sync.dma_start(XT[:, c0:c0 + bw], x[:, c0:c0 + bw])
                # ebar = (1 - 2p) / (2 lam)  (in place).  First blocks on the
                # DVE (tensor_scalar, 2x mode): the Scalar engine's queue
                # takes several us to boot, and the scan chain waits on this.
                if bi <= 1:
                    nc.vector.tensor_scalar(
                        XT[:, c0:c0 + bw], XT[:, c0:c0 + bw],
                        -2.0 / (2 * LAM), 1.0 / (2 * LAM),
                        Alu.mult, Alu.add)
                else:
                    nc.scalar.activation(XT[:, c0:c0 + bw], XT[:, c0:c0 + bw],
                                         Copy, bias=1.0 / (2 * LAM),
                                         scale=-2.0 / (2 * LAM))
                # chained running sum: SB[c+1] = SB[c] + ebar[c]
                init = 0.0 if c0 == 0 else SB[:, c0:c0 + 1]
                nc.vector.tensor_tensor_scan(
                    SB[:, c0 + 1:c0 + 1 + bw], XT[:, c0:c0 + bw],
                    XT[:, c0:c0 + bw], init, Alu.add, Alu.bypass)
                if bi <= 1:
                    nc.vector.tensor_scalar(SP[:, c0 + 1:c0 + 1 + bw],
                                            SB[:, c0 + 1:c0 + 1 + bw],
                                            1.0, None, Alu.add)
                else:
                    nc.scalar.activation(SP[:, c0 + 1:c0 + 1 + bw],
                                         SB[:, c0 + 1:c0 + 1 + bw],
                                         Copy, bias=1.0)
                # chained clamp walk: w = min(max(w, SB_excl), SP_excl)
                winit = 0.5 if c0 == 0 else WT[:, c0 - 1:c0]
                nc.vector.tensor_tensor_scan(
                    WT[:, c0:c0 + bw], SB[:, c0:c0 + bw],
                    SP[:, c0:c0 + bw], winit, Alu.max, Alu.min)

            # boundary sentinel at the last halo column:
            # w[R-1] := +-1e38 by the sign of d~ = w - SB_incl - lam~ there
            nc.scalar.activation(TMP[:], SB[:, R:R + 1], Copy, bias=0.5)
            nc.vector.tensor_tensor(CB[:], WT[:, R - 1:R], TMP[:], Alu.is_gt)
            nc.scalar.activation(WT[:, R - 1:R], CB[:], Copy,
                                 scale=2e38, bias=-1e38)

            # backtracking: label' = [w + label > SP_incl], reversed scan.
            # Emitted right-to-left; each block seeds from the label the
            # previous (righter) block produced, so chaining is exact.
            for si in range(len(BWD_BLOCKS) - 1, -1, -1):
                c0, bw = BWD_BLOCKS[si]
                last = si == len(BWD_BLOCKS) - 1
                wd = bw + (W if last else 0)
                init = 0.0 if last else LB[:, c0 + wd:c0 + wd + 1]
                nc.vector.tensor_tensor_scan(
                    LB[:, c0:c0 + wd][:, ::-1],
                    WT[:, c0:c0 + wd][:, ::-1],
                    SP[:, c0 + 1:c0 + 1 + wd][:, ::-1],
                    init, Alu.add, Alu.is_gt)
                nc.sync.dma_start(y[:, c0 - W:c0 - W + bw],
                                  LB[:, c0:c0 + bw])
    return nc


def _legalize_waits(nc, limit=1):
    """Split instructions carrying more than `limit` sem-waits.

    This walrus build rejects instructions whose sync_info has more wait
    commands than the ISA encoding allows (Tile can accumulate several).
    Excess waits move onto NoOps prepended on the same engine, which
    preserves per-engine ordering semantics.
    """
    import concourse.mybir as mybir
    for fn in nc.m.functions:
        for blk in fn.blocks:
            insts = blk.instructions
            i = 0
            while i < len(insts):
                inst = insts[i]
                si = getattr(inst, "sync_info", None)
                if si is not None and si.on_wait and len(si.on_wait) > limit:
                    waits = list(si.on_wait)
                    inst.sync_info = mybir.SyncInfo(
                        on_wait=waits[-limit:], on_update=list(si.on_update))
                    pending = waits[:-limit]
                    for j in range(0, len(pending), limit):
                        nop = mybir.InstNoOp(
                            name=nc.get_next_instruction_name(),
                            sync_info=mybir.SyncInfo(
                                on_wait=pending[j:j + limit], on_update=[]),
                            bass_nofuse=True,
                            engine=inst.engine,
                        )
                        insts.insert(i, nop)
                        i += 1
                i += 1
    return nc


_nc_cache = None


def _get_nc():
    global _nc_cache
    if _nc_cache is None:
        _nc_cache = _legalize_waits(_build())
    return _nc_cache


def _shard(inputs: np.ndarray):
    p = np.ascontiguousarray(inputs, dtype=np.float32)
    assert p.shape == (N,)
    pad = np.full(W, 0.5, np.float32)
    pp = np.concatenate([pad, p, pad])
    nrows = N // D
    X = np.lib.stride_tricks.as_strided(pp, (nrows, R), (D * 4, 4))
    return [{"x": np.ascontiguousarray(X[k * P:(k + 1) * P])}
            for k in range(NCORES)]


def _run(inputs: np.ndarray, trace: bool = False):
    in_maps = _shard(inputs)
    res = bass_utils.run_bass_kernel_spmd(_get_nc(), in_maps,
                                          core_ids=list(range(NCORES)),
                                          trace=trace)
    lab = np.concatenate([np.asarray(res.results[k]["y"]).reshape(-1)
                          for k in range(NCORES)])
    return lab.astype(np.int32), res


def kernel(inputs: np.ndarray) -> np.ndarray:
    lab, _ = _run(inputs, trace=False)
    return lab


# revision 14
# speedup vs baseline: 2.9173x; 1.0157x over previous
"""Trainium2 Bass kernel for nn_BinarizeLayer (chain Viterbi binarization).

Algorithm
---------
The reference is a 2-state Viterbi DP over an 8.4M-node chain.  Writing
d_i = a0_i - a1_i (score difference of the two states), the forward pass
collapses to the scalar recurrence

    d_i = e_i + clamp(d_{i-1}, -lam, lam),        e_i = 2*p_i - 1,

and the backtracking pass to

    label_{i-1} = label_i ? (d_{i-1} >= -lam) : (d_{i-1} > lam).

The whole problem is rescaled by 1/(2*lam) (argmins are scale
invariant), so lam~ = 0.5 and the transition cost 2*lam~ = 1 exactly.
Conjugating the clamp recurrence by prefix sums (SB = running sum of
(1-2p)/(2 lam), w = d~ + SB_incl + lam~) turns the forward pass into

    w_k = min(max(w_{k-1}, SB_k), SB_k + 1),

which is exactly the hardware `tensor_tensor_scan` (op0=max, op1=min)
over (SB, SP = SB + 1).  The backtracking pass becomes a single scan
too: label_{k-1} = [d~ > lam~ - label_k] = [w + label_k > SP_incl]
(using that the state shift is exactly 1), i.e. a reversed
`tensor_tensor_scan` with op0=add, op1=is_gt over (w, SP_incl) — no
threshold bitmaps needed at all.  (The state=1 threshold becomes
strictly-greater instead of >=, which differs only on exact fp ties.)

Engine split (per core):
  * ACT : ebar = (1-2p)/(2 lam), SP = SB + 1        (affine, 1-input)
  * DVE : SB scan, W clamp-walk scan, bwd label scan
  * Pool: unused (GpSimd streaming ops measure ~12x slower than DVE on
    this hardware and contend with DVE for SBUF ports)

Sharding: the chain is split into 8 core slices x 128 partition rows of
8192 payload elements, each row padded with a 64-element halo on both
sides (clamp recurrences forget their initial state as soon as the walk
saturates a clamp bound, so a 64-step warm-up reproduces the exact
sequential fp32 state).  The global chain ends are padded with p=0.5
(ebar = 0 exactly, incl. the rescaling, since fl(4/3)/2 == fl(2/3)).
The final-label boundary condition is injected by overwriting w at the
last halo column with +-1e38 by the sign of d~ there; backward blocks
chain exactly (descending emission, init = label just computed by the
block to the right), so backtracking is exact.
"""

import numpy as np

import concourse.bass as bass
import concourse.mybir as mybir
from concourse import tile
from concourse import bass_utils

LAM = 0.75
N = 8388608
NCORES = 8
P = 128          # partitions
W = 64           # halo / warm-up width
D = 8192         # payload elements per partition row
R = D + 2 * W    # row length incl. halos
# graded forward blocks: small first so the DVE scan chain starts ASAP and
# never outruns the input DMA stream
_FWD_W = [256, 512, 1024, 1024, 1024, 2048, 2432]
FWD_BLOCKS = []
_c = 0
for _w in _FWD_W:
    FWD_BLOCKS.append((_c, _w))
    _c += _w
assert _c == R
N_DVE_WARM = 3   # blocks whose ebar/SP run on DVE (Scalar engine boots late)
# backward blocks, emitted right-to-left; last (leftmost) small to cut the tail
_BWD_W = [256, 1792, 2048, 2048, 2048]
assert sum(_BWD_W) == D
BWD_BLOCKS = []
_c = W
for _w in _BWD_W:
    BWD_BLOCKS.append((_c, _w))
    _c += _w


def _build():
    f32 = mybir.dt.float32
    i8 = mybir.dt.int8
    Alu = mybir.AluOpType
    Copy = mybir.ActivationFunctionType.Copy

    nc = bass.Bass()
    x = nc.dram_tensor("x", [P, R], f32, kind="ExternalInput")
    y = nc.dram_tensor("y", [P, D], i8, kind="ExternalOutput")

    with tile.TileContext(nc) as tc:
        with tc.tile_pool(name="big", bufs=1) as big:
            XT = big.tile([P, R], f32)        # input p, then ebar in place
            SB = big.tile([P, R + 1], f32)    # running sum of ebar; col0 = 0
            SP = big.tile([P, R + 1], f32)    # SB + 1
            WT = XT                           # clamp walk overwrites consumed ebar
            LB = big.tile([P, R], i8)         # labels
            TMP = big.tile([P, 1], f32)
            CB = big.tile([P, 1], f32)

            nc.vector.memset(SB[:, 0:1], 0.0)
            nc.vector.memset(SP[:, 0:1], 1.0)

            for bi, (c0, bw) in enumerate(FWD_BLOCKS):
                nc.sync.dma_start(XT[:, c0:c0 + bw], x[:, c0:c0 + bw])
                # ebar = (1 - 2p) / (2 lam)  (in place).  First blocks on the
                # DVE (tensor_scalar, 2x mode): the Scalar engine's queue
                # takes several us to boot, and the scan chain waits on this.
                if bi < N_DVE_WARM:
                    nc.vector.tensor_scalar(
                        XT[:, c0:c0 + bw], XT[:, c0:c0 + bw],
                        -2.0 / (2 * LAM), 1.0 / (2 * LAM),
                        Alu.mult, Alu.add)
                else:
                    nc.scalar.activation(XT[:, c0:c0 + bw], XT[:, c0:c0 + bw],
                                         Copy, bias=1.0 / (2 * LAM),
                                         scale=-2.0 / (2 * LAM))
                # chained running sum: SB[c+1] = SB[c] + ebar[c]
                init = 0.0 if c0 == 0 else SB[:, c0:c0 + 1]
                nc.vector.tensor_tensor_scan(
                    SB[:, c0 + 1:c0 + 1 + bw], XT[:, c0:c0 + bw],
                    XT[:, c0:c0 + bw], init, Alu.add, Alu.bypass)
                if bi < N_DVE_WARM:
                    nc.vector.tensor_scalar(SP[:, c0 + 1:c0 + 1 + bw],
                                            SB[:, c0 + 1:c0 + 1 + bw],
                                            1.0, None, Alu.add)
                else:
                    nc.scalar.activation(SP[:, c0 + 1:c0 + 1 + bw],
                                         SB[:, c0 + 1:c0 + 1 + bw],
                                         Copy, bias=1.0)
                # chained clamp walk: w = min(max(w, SB_excl), SP_excl)
                winit = 0.5 if c0 == 0 else WT[:, c0 - 1:c0]
                nc.vector.tensor_tensor_scan(
                    WT[:, c0:c0 + bw], SB[:, c0:c0 + bw],
                    SP[:, c0:c0 + bw], winit, Alu.max, Alu.min)

            # boundary sentinel at the last halo column:
            # w[R-1] := +-1e38 by the sign of d~ = w - SB_incl - lam~ there
            nc.scalar.activation(TMP[:], SB[:, R:R + 1], Copy, bias=0.5)
            nc.vector.tensor_tensor(CB[:], WT[:, R - 1:R], TMP[:], Alu.is_gt)
            nc.scalar.activation(WT[:, R - 1:R], CB[:], Copy,
                                 scale=2e38, bias=-1e38)

            # backtracking: label' = [w + label > SP_incl], reversed scan.
            # Emitted right-to-left; each block seeds from the label the
            # previous (righter) block produced, so chaining is exact.
            for si in range(len(BWD_BLOCKS) - 1, -1, -1):
                c0, bw = BWD_BLOCKS[si]
                last = si == len(BWD_BLOCKS) - 1
                wd = bw + (W if last else 0)
                init = 0.0 if last else LB[:, c0 + wd:c0 + wd + 1]
                nc.vector.tensor_tensor_scan(
                    LB[:, c0:c0 + wd][:, ::-1],
                    WT[:, c0:c0 + wd][:, ::-1],
                    SP[:, c0 + 1:c0 + 1 + wd][:, ::-1],
                    init, Alu.add, Alu.is_gt)
                nc.sync.dma_start(y[:, c0 - W:c0 - W + bw],
                                  LB[:, c0:c0 + bw])
    return nc


def _legalize_waits(nc, limit=1):
    """Split instructions carrying more than `limit` sem-waits.

    This walrus build rejects instructions whose sync_info has more wait
    commands than the ISA encoding allows (Tile can accumulate several).
    Excess waits move onto NoOps prepended on the same engine, which
    preserves per-engine ordering semantics.
    """
    import concourse.mybir as mybir
    for fn in nc.m.functions:
        for blk in fn.blocks:
            insts = blk.instructions
            i = 0
            while i < len(insts):
                inst = insts[i]
                si = getattr(inst, "sync_info", None)
                if si is not None and si.on_wait and len(si.on_wait) > limit:
                    waits = list(si.on_wait)
                    inst.sync_info = mybir.SyncInfo(
                        on_wait=waits[-limit:], on_update=list(si.on_update))
                    pending = waits[:-limit]
                    for j in range(0, len(pending), limit):
                        nop = mybir.InstNoOp(
                            name=nc.get_next_instruction_name(),
                            sync_info=mybir.SyncInfo(
                                on_wait=pending[j:j + limit], on_update=[]),
                            bass_nofuse=True,
                            engine=inst.engine,
                        )
                        insts.insert(i, nop)
                        i += 1
                i += 1
    return nc


_nc_cache = None


def _get_nc():
    global _nc_cache
    if _nc_cache is None:
        _nc_cache = _legalize_waits(_build())
    return _nc_cache


def _shard(inputs: np.ndarray):
    p = np.ascontiguousarray(inputs, dtype=np.float32)
    assert p.shape == (N,)
    pad = np.full(W, 0.5, np.float32)
    pp = np.concatenate([pad, p, pad])
    nrows = N // D
    X = np.lib.stride_tricks.as_strided(pp, (nrows, R), (D * 4, 4))
    return [{"x": np.ascontiguousarray(X[k * P:(k + 1) * P])}
            for k in range(NCORES)]


def _run(inputs: np.ndarray, trace: bool = False):
    in_maps = _shard(inputs)
    res = bass_utils.run_bass_kernel_spmd(_get_nc(), in_maps,
                                          core_ids=list(range(NCORES)),
                                          trace=trace)
    lab = np.concatenate([np.asarray(res.results[k]["y"]).reshape(-1)
                          for k in range(NCORES)])
    return lab.astype(np.int32), res


def kernel(inputs: np.ndarray) -> np.ndarray:
    lab, _ = _run(inputs, trace=False)
    return lab
